# revision 16
# baseline (speedup 1.0000x reference)
"""Trainium2 Bass kernel for nn_Conv_DCFD (dynamic conv filter decomposition).

Data-parallel over batch N=8 across 8 NeuronCores (one sample per core).

Per-sample pipeline (all shapes hardcoded):
  D. Y_T per block: x_blk.T @ coef -> yvs [128px, M, O] bf16 (emitted first so
     the psum->sbuf casts overlap conv matmuls on PE)
  A. conv1 3x3 (C=128 -> 64) + folded BN + tanh      [PE tap-loop, f32r]
  B. conv2 3x3 (64 -> 72) + folded BN + tanh -> h2 bf16
  C. basesT per 128-px block: h2_blk.T @ FBBD (bf16) -> [128px, M, 2, 26]
  E. per (pair, 2m): GPSIMD local_scatter builds banded A^T rows; DMA-XBAR
     transposes flip each [128,128] chunk into a3 (SBUF) without touching
     PE or PSUM; out_T[block] += a3_b.T @ yvs[block+b-1] accumulated in PSUM.
  F. po -> obuf [128, NBLK, O]; one DMA per 8 blocks; host transposes + bias.
"""

import numpy as np
import ml_dtypes

N, C, H, W = 8, 128, 64, 64
O, KS, M, TEM, BS, INTER = 128, 5, 6, 12, 72, 64
EPS = 1e-5
PIX = H * W
NBLK = PIX // 128

_f32 = np.float32
_bf16 = ml_dtypes.bfloat16

_cached = {}


def _host_prep(inputs):
    """Fold BN, rearrange weights; returns dict of per-core-shared arrays."""
    conv1_w = np.asarray(inputs["conv1_w"], _f32)
    conv1_b = np.asarray(inputs["conv1_b"], _f32)
    conv2_w = np.asarray(inputs["conv2_w"], _f32)
    conv2_b = np.asarray(inputs["conv2_b"], _f32)
    fb = np.asarray(inputs["fb_bases"], _f32)
    coef = np.asarray(inputs["coef"], _f32)

    s1 = np.asarray(inputs["bn1_gamma"], _f32) / np.sqrt(np.asarray(inputs["bn1_var"], _f32) + EPS)
    t1 = (conv1_b - np.asarray(inputs["bn1_mean"], _f32)) * s1 + np.asarray(inputs["bn1_beta"], _f32)
    s2 = np.asarray(inputs["bn2_gamma"], _f32) / np.sqrt(np.asarray(inputs["bn2_var"], _f32) + EPS)
    t2 = (conv2_b - np.asarray(inputs["bn2_mean"], _f32)) * s2 + np.asarray(inputs["bn2_beta"], _f32)

    w1T = np.ascontiguousarray(np.transpose(conv1_w.reshape(INTER, C, 9), (1, 2, 0)))  # [C,9,INTER]
    w2T = np.ascontiguousarray(np.transpose(conv2_w.reshape(BS, INTER, 9), (1, 2, 0)))  # [INTER,9,BS]

    FBBD = np.zeros((BS, M * 25), _f32)
    for m in range(M):
        FBBD[m * TEM:(m + 1) * TEM, m * 25:(m + 1) * 25] = fb

    coefT = np.zeros((C, M, O), _f32)
    for m in range(M):
        coefT[:, m, :] = coef[:, m::M].T

    idx = np.full((128, 26), -1, np.int16)
    for i in range(128):
        col = i % 64
        for dy in range(-2, 3):
            for dx in range(-2, 3):
                if 0 <= col + dx < 64:
                    idx[i, (dy + 2) * 5 + (dx + 2)] = i + 64 * dy + dx + 128
    idx2 = np.full((128, 52), -1, np.int16)
    idx2[:, 0:26] = idx
    idx2[:, 26:52] = np.where(idx >= 0, idx + 384, -1)
    idx4 = np.full((128, 104), -1, np.int16)
    idx4[:, 0:52] = idx2
    idx4[:, 52:104] = np.where(idx2 >= 0, idx2 + 768, -1)

    return {
        "w1T": w1T,
        "s1": s1.reshape(INTER, 1),
        "t1": t1.reshape(INTER, 1),
        "w2T": w2T,
        "s2": s2.reshape(BS, 1),
        "t2": t2.reshape(BS, 1),
        "fbbd": FBBD.astype(_bf16),
        "coefT": coefT.astype(_bf16),
        "idx4": idx4,
        "ident": np.eye(128, dtype=_bf16),
        "bias": np.asarray(inputs["bias"], _f32),
    }


def _build_program():
    import concourse.bass as bass
    import concourse.mybir as mybir
    import concourse.tile as tile
    from concourse import bacc

    f32 = mybir.dt.float32
    f32r = mybir.dt.float32r
    bf16 = mybir.dt.bfloat16
    i16 = mybir.dt.int16
    Tanh = mybir.ActivationFunctionType.Tanh

    nc = bacc.Bacc("TRN2", target_bir_lowering=False, debug=False, num_devices=8)

    xp_d = nc.dram_tensor("xp", [C, 66 * 66], f32r, kind="ExternalInput").ap()
    xbf_d = nc.dram_tensor("xbf", [C, PIX], bf16, kind="ExternalInput").ap()
    w1_d = nc.dram_tensor("w1t", [C, 9 * INTER], f32r, kind="ExternalInput").ap()
    s1_d = nc.dram_tensor("s1", [INTER, 1], f32, kind="ExternalInput").ap()
    t1_d = nc.dram_tensor("t1", [INTER, 1], f32, kind="ExternalInput").ap()
    w2_d = nc.dram_tensor("w2t", [INTER, 9 * BS], f32r, kind="ExternalInput").ap()
    s2_d = nc.dram_tensor("s2", [BS, 1], f32, kind="ExternalInput").ap()
    t2_d = nc.dram_tensor("t2", [BS, 1], f32, kind="ExternalInput").ap()
    fbbd_d = nc.dram_tensor("fbbd", [BS, M * 25], bf16, kind="ExternalInput").ap()
    coef_d = nc.dram_tensor("coeft", [C, M * O], bf16, kind="ExternalInput").ap()
    idx_d = nc.dram_tensor("idx4", [128, 104], i16, kind="ExternalInput").ap()
    ident_d = nc.dram_tensor("ident", [128, 128], bf16, kind="ExternalInput").ap()
    out_d = nc.dram_tensor("out", [PIX, O], f32, kind="ExternalOutput").ap()

    taps = [(a, b) for a in range(3) for b in range(3)]

    from contextlib import ExitStack

    with tile.TileContext(nc) as tc, ExitStack() as stack:
        consts = stack.enter_context(tc.tile_pool(name="consts", bufs=1))
        apool = stack.enter_context(tc.tile_pool(name="apool", bufs=3))
        a3pool = stack.enter_context(tc.tile_pool(name="a3pool", bufs=26))

        # ---- load constants / inputs into SBUF (chunked + spread so the
        # first consumers unblock early) ----
        xbf = consts.tile([C, NBLK, 128], bf16)
        xbf_src = xbf_d.rearrange("c (b p) -> c b p", b=NBLK)
        for c0 in range(0, NBLK, 8):
            nc.sync.dma_start(out=xbf[:, c0:c0 + 8, :], in_=xbf_src[:, c0:c0 + 8, :])
        coefT = consts.tile([C, M, O], bf16)
        nc.sync.dma_start(out=coefT, in_=coef_d.rearrange("c (m o) -> c m o", m=M))
        xp = consts.tile([C, 66, 66], f32r)
        xp_src = xp_d.rearrange("c (h w) -> c h w", h=66)
        for r0, r1 in ((0, 17), (17, 33), (33, 66)):
            nc.scalar.dma_start(out=xp[:, r0:r1, :], in_=xp_src[:, r0:r1, :])
        w1 = consts.tile([C, 9, INTER], f32r)
        nc.gpsimd.dma_start(out=w1, in_=w1_d.rearrange("c (t o) -> c t o", t=9))
        w2 = consts.tile([INTER, 9, BS], f32r)
        nc.gpsimd.dma_start(out=w2, in_=w2_d.rearrange("c (t o) -> c t o", t=9))
        s1 = consts.tile([INTER, 1], f32)
        nc.gpsimd.dma_start(out=s1, in_=s1_d)
        t1 = consts.tile([INTER, 1], f32)
        nc.gpsimd.dma_start(out=t1, in_=t1_d)
        s2 = consts.tile([BS, 1], f32)
        nc.gpsimd.dma_start(out=s2, in_=s2_d)
        t2 = consts.tile([BS, 1], f32)
        nc.gpsimd.dma_start(out=t2, in_=t2_d)
        fbbd = consts.tile([BS, M * 25], bf16)
        nc.gpsimd.dma_start(out=fbbd, in_=fbbd_d)
        idxt = consts.tile([128, 104], i16)
        nc.gpsimd.dma_start(out=idxt, in_=idx_d)
        ident = consts.tile([128, 128], bf16)
        nc.gpsimd.dma_start(out=ident, in_=ident_d)

        h1p = consts.tile([INTER, 66, 66], f32r)
        h2 = consts.tile([BS, PIX], bf16)
        basesT = consts.tile([128, NBLK // 2, M, 2, 26], bf16)
        yvs = consts.tile([128, NBLK, M, O], bf16)
        obuf = consts.tile([128, NBLK, O], f32)
        zero_y = consts.tile([128, M, O], bf16)
        nc.vector.memset(zero_y, 0.0)
        # zero h1p borders (interior overwritten by conv1)
        nc.vector.memset(h1p[:, 0, :].bitcast(f32), 0.0)
        nc.vector.memset(h1p[:, 65, :].bitcast(f32), 0.0)
        nc.vector.memset(h1p[:, :, 0].bitcast(f32), 0.0)
        nc.vector.memset(h1p[:, :, 65].bitcast(f32), 0.0)

        psA = stack.enter_context(tc.tile_pool(name="psA", bufs=2, space="PSUM"))
        psB = stack.enter_context(tc.tile_pool(name="psB", bufs=1, space="PSUM"))
        psT = stack.enter_context(tc.tile_pool(name="psT", bufs=2, space="PSUM"))
        psY = stack.enter_context(tc.tile_pool(name="psY", bufs=2, space="PSUM"))
        psO = stack.enter_context(tc.tile_pool(name="psO", bufs=1, space="PSUM"))

        def d_chunk(b0, b1):
            # Y_T for blocks [b0, b1): casts overlap PE work elsewhere
            for B in range(b0, b1):
                for h in range(2):
                    py = psY.tile([128, 3 * O], f32, tag="py")
                    nc.tensor.matmul(
                        py[:],
                        lhsT=xbf[:, B, :],
                        rhs=coefT[:, 3 * h: 3 * h + 3, :].rearrange("c m o -> c (m o)"),
                        start=True,
                        stop=True,
                    )
                    dst = yvs[:, B, 3 * h: 3 * h + 3, :].rearrange("p m o -> p (m o)")
                    if (2 * B + h) % 2 == 0:
                        nc.vector.tensor_copy(dst, py[:])
                    else:
                        nc.scalar.copy(dst, py[:])

        # ---- helpers ----
        a3s = [[None] * NBLK for _ in range(M)]
        zero_ap = zero_y[:]

        def ytv(i):
            # y tile for band source block index i-1 (i in 0..NBLK+1)
            if i == 0 or i == NBLK + 1:
                return zero_ap
            return yvs[:, i - 1]

        def conv1(r):
            p1 = psA.tile([INTER, 512], f32, tag="conv")
            for t, (a, b) in enumerate(taps):
                nc.tensor.matmul(
                    p1[:],
                    lhsT=w1[:, t, :],
                    rhs=xp[:, a + 8 * r: a + 8 * r + 8, b: b + 64],
                    start=(t == 0),
                    stop=(t == 8),
                )
            nc.scalar.activation(
                h1p[:, 1 + 8 * r: 9 + 8 * r, 1:65],
                p1[:].rearrange("p (a b) -> p a b", a=8),
                Tanh,
                bias=t1[:],
                scale=s1[:],
            )

        def conv2(r):
            p2 = psA.tile([BS, 512], f32, tag="conv")
            for t, (a, b) in enumerate(taps):
                nc.tensor.matmul(
                    p2[:],
                    lhsT=w2[:, t, :],
                    rhs=h1p[:, a + 8 * r: a + 8 * r + 8, b: b + 64],
                    start=(t == 0),
                    stop=(t == 8),
                )
            nc.scalar.activation(
                h2[:, 512 * r: 512 * (r + 1)],
                p2[:],
                Tanh,
                bias=t2[:],
                scale=s2[:],
            )

        def do_C(B):
            pb = psB.tile([128, M * 25], f32, tag="pb")
            nc.tensor.matmul(
                pb[:],
                lhsT=h2[:, 128 * B: 128 * (B + 1)],
                rhs=fbbd[:],
                start=True,
                stop=True,
            )
            dst = basesT[:, B // 2, :, B % 2, 0:25]
            src = pb[:].rearrange("p (m l) -> p m l", m=M)
            if B % 2 == 0:
                nc.vector.tensor_copy(dst, src)
            else:
                nc.scalar.copy(dst, src)

        def build_pair(B):
            # banded matrices for pair (B-1, B): scatter 2 m at a time, PE
            # transposes into one PSUM tile, single copy per a3
            P = B // 2
            for mb in range(3):
                at2 = apool.tile([128, 1536], bf16, tag="at")
                nc.gpsimd.local_scatter(
                    at2[:],
                    basesT[:, P, 2 * mb: 2 * mb + 2, :, :].rearrange(
                        "p m b l -> p (m b l)"),
                    idxt[:],
                    channels=128,
                    num_elems=1536,
                    num_idxs=104,
                )
                for dm in range(2):
                    m = 2 * mb + dm
                    for half in range(2):
                        pt = psT.tile([128, 3, 128], bf16, tag="pt")
                        base = dm * 768 + half * 384
                        for b in range(3):
                            nc.tensor.transpose(
                                pt[:, b, :],
                                at2[:, base + 128 * b: base + 128 * (b + 1)],
                                ident[:],
                            )
                        a3 = a3pool.tile([128, 3, 128], bf16, tag="a3")
                        if (2 * m + half) % 3 != 2:
                            nc.vector.tensor_copy(a3[:], pt[:])
                        else:
                            nc.scalar.copy(a3[:], pt[:])
                        a3s[m][B - 1 + half] = a3

        def emit_banded(B):
            po = psO.tile([128, O], f32, tag="po")
            for m in range(M):
                a3 = a3s[m][B]
                for b in range(3):
                    nc.tensor.matmul(
                        po[:],
                        lhsT=a3[:, b, :],
                        rhs=ytv(B + b)[:, m, :],
                        start=(m == 0 and b == 0),
                        stop=(m == M - 1 and b == 2),
                    )
            nc.vector.tensor_copy(obuf[:, B, :], po[:])
            if B % 8 == 7:
                k = B // 8
                nc.sync.dma_start(
                    out=out_d[1024 * k: 1024 * (k + 1), :].rearrange(
                        "(b p) o -> p b o", p=128),
                    in_=obuf[:, 8 * k: 8 * k + 8, :],
                )

        # ---- interleaved schedule: preroll gets the first scatters going
        # ASAP, then conv/C/pair/emit per row-chunk with D blocks as PE
        # filler after each chunk's GPSIMD dependencies are emitted ----
        d_chunk(0, 8)
        conv1(0)
        conv1(1)
        conv2(0)
        for B in range(4):
            do_C(B)
            if B % 2 == 1:
                build_pair(B)
        for r in range(1, 8):
            if r + 1 < 8:
                conv1(r + 1)
            conv2(r)
            if r == 1:
                emit_banded(0)
                emit_banded(1)
            for B in range(4 * r, 4 * r + 4):
                do_C(B)
                if B % 2 == 1:
                    build_pair(B)
                if B >= 2:
                    emit_banded(B - 2)
                    if B == NBLK - 1:
                        emit_banded(B - 1)
                        emit_banded(B)
            if r <= 6:
                d_chunk(8 + 4 * (r - 1), 12 + 4 * (r - 1))

    nc.compile()
    return nc


def _get_program():
    if "nc" not in _cached:
        _cached["nc"] = _build_program()
    return _cached["nc"]


def _build_in_maps(inputs, prep=None):
    if prep is None:
        prep = _host_prep(inputs)
    x = np.asarray(inputs["x"], _f32)

    shared = {
        "w1t": np.ascontiguousarray(prep["w1T"].reshape(C, 9 * INTER)),
        "s1": prep["s1"], "t1": prep["t1"],
        "w2t": np.ascontiguousarray(prep["w2T"].reshape(INTER, 9 * BS)),
        "s2": prep["s2"], "t2": prep["t2"],
        "fbbd": prep["fbbd"],
        "coeft": np.ascontiguousarray(prep["coefT"].reshape(C, M * O)),
        "idx4": prep["idx4"],
        "ident": prep["ident"],
    }

    in_maps = []
    for n in range(N):
        xn = x[n]
        xpad = np.zeros((C, 66, 66), _f32)
        xpad[:, 1:65, 1:65] = xn
        m = dict(shared)
        m["xp"] = xpad.reshape(C, 66 * 66)
        m["xbf"] = np.ascontiguousarray(xn.reshape(C, PIX).astype(_bf16))
        in_maps.append(m)
    return in_maps


def kernel(**inputs):
    from concourse.bass_utils import run_bass_kernel_spmd

    prep = _host_prep(inputs)
    in_maps = _build_in_maps(inputs, prep)

    nc = _get_program()
    res = run_bass_kernel_spmd(nc, in_maps, core_ids=list(range(N)))

    out = np.zeros((N, O, H, W), _f32)
    bias = prep["bias"]
    for n in range(N):
        outT = res.results[n]["out"]            # [4096, 128]
        out[n] = (outT.T + bias[:, None]).reshape(O, H, W)
    return out


# revision 25
# speedup vs baseline: 1.0681x; 1.0681x over previous
"""Trainium2 Bass kernel for nn_Conv_DCFD (dynamic conv filter decomposition).

Data-parallel over batch N=8 across 8 NeuronCores (one sample per core).

Per-sample pipeline (all shapes hardcoded):
  D. Y_T per block: x_blk.T @ coef -> yvs [128px, M, O] bf16 (emitted first so
     the psum->sbuf casts overlap conv matmuls on PE)
  A. conv1 3x3 (C=128 -> 64) + folded BN + tanh      [PE tap-loop, f32r]
  B. conv2 3x3 (64 -> 72) + folded BN + tanh -> h2 bf16
  C. basesT per 128-px block: h2_blk.T @ FBBD (bf16) -> [128px, M, 2, 26]
  E. per (pair, 2m): GPSIMD local_scatter builds banded A^T rows; DMA-XBAR
     transposes flip each [128,128] chunk into a3 (SBUF) without touching
     PE or PSUM; out_T[block] += a3_b.T @ yvs[block+b-1] accumulated in PSUM.
  F. po -> obuf [128, NBLK, O]; one DMA per 8 blocks; host transposes + bias.
"""

import numpy as np
import ml_dtypes

N, C, H, W = 8, 128, 64, 64
O, KS, M, TEM, BS, INTER = 128, 5, 6, 12, 72, 64
EPS = 1e-5
PIX = H * W
NBLK = PIX // 128

_f32 = np.float32
_bf16 = ml_dtypes.bfloat16

_cached = {}


def _host_prep(inputs):
    """Fold BN, rearrange weights; returns dict of per-core-shared arrays."""
    conv1_w = np.asarray(inputs["conv1_w"], _f32)
    conv1_b = np.asarray(inputs["conv1_b"], _f32)
    conv2_w = np.asarray(inputs["conv2_w"], _f32)
    conv2_b = np.asarray(inputs["conv2_b"], _f32)
    fb = np.asarray(inputs["fb_bases"], _f32)
    coef = np.asarray(inputs["coef"], _f32)

    s1 = np.asarray(inputs["bn1_gamma"], _f32) / np.sqrt(np.asarray(inputs["bn1_var"], _f32) + EPS)
    t1 = (conv1_b - np.asarray(inputs["bn1_mean"], _f32)) * s1 + np.asarray(inputs["bn1_beta"], _f32)
    s2 = np.asarray(inputs["bn2_gamma"], _f32) / np.sqrt(np.asarray(inputs["bn2_var"], _f32) + EPS)
    t2 = (conv2_b - np.asarray(inputs["bn2_mean"], _f32)) * s2 + np.asarray(inputs["bn2_beta"], _f32)

    w1T = np.ascontiguousarray(
        np.transpose(conv1_w.reshape(INTER, C, 9), (1, 2, 0))).astype(_bf16)  # [C,9,INTER]
    w2T = np.ascontiguousarray(np.transpose(conv2_w.reshape(BS, INTER, 9), (1, 2, 0)))  # [INTER,9,BS]

    FBBD = np.zeros((BS, M * 25), _f32)
    for m in range(M):
        FBBD[m * TEM:(m + 1) * TEM, m * 25:(m + 1) * 25] = fb

    coefT = np.zeros((C, M, O), _f32)
    for m in range(M):
        coefT[:, m, :] = coef[:, m::M].T

    idx = np.full((128, 26), -1, np.int16)
    for i in range(128):
        col = i % 64
        for dy in range(-2, 3):
            for dx in range(-2, 3):
                if 0 <= col + dx < 64:
                    idx[i, (dy + 2) * 5 + (dx + 2)] = i + 64 * dy + dx + 128
    idx2 = np.full((128, 52), -1, np.int16)
    idx2[:, 0:26] = idx
    idx2[:, 26:52] = np.where(idx >= 0, idx + 384, -1)
    idx4 = np.full((128, 104), -1, np.int16)
    idx4[:, 0:52] = idx2
    idx4[:, 52:104] = np.where(idx2 >= 0, idx2 + 768, -1)

    return {
        "w1T": w1T,
        "s1": s1.reshape(INTER, 1),
        "t1": t1.reshape(INTER, 1),
        "w2T": w2T,
        "s2": s2.reshape(BS, 1),
        "t2": t2.reshape(BS, 1),
        "fbbd": FBBD.astype(_bf16),
        "coefT": coefT.astype(_bf16),
        "idx4": idx4,
        "ident": np.eye(128, dtype=_bf16),
        "bias": np.asarray(inputs["bias"], _f32),
    }


def _build_program():
    import concourse.bass as bass
    import concourse.mybir as mybir
    import concourse.tile as tile
    from concourse import bacc

    f32 = mybir.dt.float32
    f32r = mybir.dt.float32r
    bf16 = mybir.dt.bfloat16
    i16 = mybir.dt.int16
    Tanh = mybir.ActivationFunctionType.Tanh

    nc = bacc.Bacc("TRN2", target_bir_lowering=False, debug=False, num_devices=8)

    xbf_d = nc.dram_tensor("xbf", [C, PIX], bf16, kind="ExternalInput").ap()
    w1_d = nc.dram_tensor("w1t", [C, 9 * INTER], bf16, kind="ExternalInput").ap()
    s1_d = nc.dram_tensor("s1", [INTER, 1], f32, kind="ExternalInput").ap()
    t1_d = nc.dram_tensor("t1", [INTER, 1], f32, kind="ExternalInput").ap()
    w2_d = nc.dram_tensor("w2t", [INTER, 9 * BS], f32r, kind="ExternalInput").ap()
    s2_d = nc.dram_tensor("s2", [BS, 1], f32, kind="ExternalInput").ap()
    t2_d = nc.dram_tensor("t2", [BS, 1], f32, kind="ExternalInput").ap()
    fbbd_d = nc.dram_tensor("fbbd", [BS, M * 25], bf16, kind="ExternalInput").ap()
    coef_d = nc.dram_tensor("coeft", [C, M * O], bf16, kind="ExternalInput").ap()
    idx_d = nc.dram_tensor("idx4", [128, 104], i16, kind="ExternalInput").ap()
    ident_d = nc.dram_tensor("ident", [128, 128], bf16, kind="ExternalInput").ap()
    out_d = nc.dram_tensor("out", [PIX, O], f32, kind="ExternalOutput").ap()

    taps = [(a, b) for a in range(3) for b in range(3)]

    from contextlib import ExitStack

    with tile.TileContext(nc) as tc, ExitStack() as stack:
        consts = stack.enter_context(tc.tile_pool(name="consts", bufs=1))
        apool = stack.enter_context(tc.tile_pool(name="apool", bufs=6))
        a3pool = stack.enter_context(tc.tile_pool(name="a3pool", bufs=30))

        # ---- load constants / inputs into SBUF (chunked + spread so the
        # first consumers unblock early) ----
        xbf = consts.tile([C, NBLK, 128], bf16)
        xbf_src = xbf_d.rearrange("c (b p) -> c b p", b=NBLK)
        for c0 in range(0, NBLK, 8):
            nc.sync.dma_start(out=xbf[:, c0:c0 + 8, :], in_=xbf_src[:, c0:c0 + 8, :])
        coefT = consts.tile([C, M, O], bf16)
        nc.sync.dma_start(out=coefT, in_=coef_d.rearrange("c (m o) -> c m o", m=M))
        w1 = consts.tile([C, 9, INTER], bf16)
        nc.gpsimd.dma_start(out=w1, in_=w1_d.rearrange("c (t o) -> c t o", t=9))
        # padded conv1 input built from xbf via SBUF->SBUF DMA (no extra HBM
        # fetch); borders zeroed below
        xp = consts.tile([C, 66, 66], bf16)
        for k in range(4):
            nc.scalar.dma_start(
                out=xp[:, 1 + 16 * k: 17 + 16 * k, 1:65],
                in_=xbf[:, 8 * k: 8 * k + 8, :].rearrange(
                    "c b (r w) -> c (b r) w", r=2),
            )
        w2 = consts.tile([INTER, 9, BS], f32r)
        nc.gpsimd.dma_start(out=w2, in_=w2_d.rearrange("c (t o) -> c t o", t=9))
        s1 = consts.tile([INTER, 1], f32)
        nc.gpsimd.dma_start(out=s1, in_=s1_d)
        t1 = consts.tile([INTER, 1], f32)
        nc.gpsimd.dma_start(out=t1, in_=t1_d)
        s2 = consts.tile([BS, 1], f32)
        nc.gpsimd.dma_start(out=s2, in_=s2_d)
        t2 = consts.tile([BS, 1], f32)
        nc.gpsimd.dma_start(out=t2, in_=t2_d)
        fbbd = consts.tile([BS, M * 25], bf16)
        nc.gpsimd.dma_start(out=fbbd, in_=fbbd_d)
        idxt = consts.tile([128, 104], i16)
        nc.gpsimd.dma_start(out=idxt, in_=idx_d)
        ident = consts.tile([128, 128], bf16)
        nc.gpsimd.dma_start(out=ident, in_=ident_d)

        h1p = consts.tile([INTER, 66, 66], f32r)
        h2 = consts.tile([BS, PIX], bf16)
        basesT = consts.tile([128, NBLK // 2, M, 2, 26], bf16)
        yvs = consts.tile([128, NBLK, M, O], bf16)
        obuf = consts.tile([128, NBLK, O], f32)
        zero_y = consts.tile([128, M, O], bf16)
        nc.vector.memset(zero_y, 0.0)
        # zero h1p borders (interior overwritten by conv1)
        nc.vector.memset(h1p[:, 0, :].bitcast(f32), 0.0)
        nc.vector.memset(h1p[:, 65, :].bitcast(f32), 0.0)
        nc.vector.memset(h1p[:, :, 0].bitcast(f32), 0.0)
        nc.vector.memset(h1p[:, :, 65].bitcast(f32), 0.0)
        # zero xp borders: rows 0/65, plus the adjacent (r,65),(r+1,0) bf16
        # pairs which are contiguous in memory (one f32 each)
        nc.vector.memset(xp[:, 0, :], 0.0)
        nc.vector.memset(xp[:, 65, :], 0.0)
        xpf = xp[:].rearrange("c h w -> c (h w)")
        xpairs = xpf[:, 65:65 + 65 * 66].rearrange("c (r t) -> c r t", t=66)
        nc.vector.memset(xpairs[:, :, 0:2], 0.0)

        psA = stack.enter_context(tc.tile_pool(name="psA", bufs=2, space="PSUM"))
        psB = stack.enter_context(tc.tile_pool(name="psB", bufs=1, space="PSUM"))
        psT = stack.enter_context(tc.tile_pool(name="psT", bufs=2, space="PSUM"))
        psY = stack.enter_context(tc.tile_pool(name="psY", bufs=2, space="PSUM"))
        psO = stack.enter_context(tc.tile_pool(name="psO", bufs=1, space="PSUM"))

        def d_chunk(b0, b1):
            # Y_T for blocks [b0, b1): casts overlap PE work elsewhere
            for B in range(b0, b1):
                for h in range(2):
                    py = psY.tile([128, 3 * O], f32, tag="py")
                    nc.tensor.matmul(
                        py[:],
                        lhsT=xbf[:, B, :],
                        rhs=coefT[:, 3 * h: 3 * h + 3, :].rearrange("c m o -> c (m o)"),
                        start=True,
                        stop=True,
                    )
                    dst = yvs[:, B, 3 * h: 3 * h + 3, :].rearrange("p m o -> p (m o)")
                    if (2 * B + h) % 2 == 0:
                        nc.vector.tensor_copy(dst, py[:])
                    else:
                        nc.scalar.copy(dst, py[:])

        # ---- helpers ----
        a3s = [[None] * NBLK for _ in range(M)]
        zero_ap = zero_y[:]

        def ytv(i):
            # y tile for band source block index i-1 (i in 0..NBLK+1)
            if i == 0 or i == NBLK + 1:
                return zero_ap
            return yvs[:, i - 1]

        def conv1(r):
            p1 = psA.tile([INTER, 512], f32, tag="conv")
            for t, (a, b) in enumerate(taps):
                nc.tensor.matmul(
                    p1[:],
                    lhsT=w1[:, t, :],
                    rhs=xp[:, a + 8 * r: a + 8 * r + 8, b: b + 64],
                    start=(t == 0),
                    stop=(t == 8),
                )
            nc.scalar.activation(
                h1p[:, 1 + 8 * r: 9 + 8 * r, 1:65],
                p1[:].rearrange("p (a b) -> p a b", a=8),
                Tanh,
                bias=t1[:],
                scale=s1[:],
            )

        def conv2(r):
            p2 = psA.tile([BS, 512], f32, tag="conv")
            for t, (a, b) in enumerate(taps):
                nc.tensor.matmul(
                    p2[:],
                    lhsT=w2[:, t, :],
                    rhs=h1p[:, a + 8 * r: a + 8 * r + 8, b: b + 64],
                    start=(t == 0),
                    stop=(t == 8),
                )
            nc.scalar.activation(
                h2[:, 512 * r: 512 * (r + 1)],
                p2[:],
                Tanh,
                bias=t2[:],
                scale=s2[:],
            )

        def do_C(B):
            pb = psB.tile([128, M * 25], f32, tag="pb")
            nc.tensor.matmul(
                pb[:],
                lhsT=h2[:, 128 * B: 128 * (B + 1)],
                rhs=fbbd[:],
                start=True,
                stop=True,
            )
            dst = basesT[:, B // 2, :, B % 2, 0:25]
            src = pb[:].rearrange("p (m l) -> p m l", m=M)
            if B % 2 == 0:
                nc.vector.tensor_copy(dst, src)
            else:
                nc.scalar.copy(dst, src)

        def build_pair(B):
            # banded matrices for pair (B-1, B): scatter 2 m at a time, PE
            # transposes into one PSUM tile, single copy per a3
            P = B // 2
            for mb in range(3):
                at2 = apool.tile([128, 1536], bf16, tag="at")
                nc.gpsimd.local_scatter(
                    at2[:],
                    basesT[:, P, 2 * mb: 2 * mb + 2, :, :].rearrange(
                        "p m b l -> p (m b l)"),
                    idxt[:],
                    channels=128,
                    num_elems=1536,
                    num_idxs=104,
                )
                for dm in range(2):
                    m = 2 * mb + dm
                    for half in range(2):
                        pt = psT.tile([128, 3, 128], bf16, tag="pt")
                        base = dm * 768 + half * 384
                        for b in range(3):
                            nc.tensor.transpose(
                                pt[:, b, :],
                                at2[:, base + 128 * b: base + 128 * (b + 1)],
                                ident[:],
                            )
                        a3 = a3pool.tile([128, 3, 128], bf16, tag="a3")
                        if (2 * m + half) % 3 != 2:
                            nc.vector.tensor_copy(a3[:], pt[:])
                        else:
                            nc.scalar.copy(a3[:], pt[:])
                        a3s[m][B - 1 + half] = a3

        def emit_banded(B):
            po = psO.tile([128, O], f32, tag="po")
            for m in range(M):
                a3 = a3s[m][B]
                for b in range(3):
                    nc.tensor.matmul(
                        po[:],
                        lhsT=a3[:, b, :],
                        rhs=ytv(B + b)[:, m, :],
                        start=(m == 0 and b == 0),
                        stop=(m == M - 1 and b == 2),
                    )
            nc.vector.tensor_copy(obuf[:, B, :], po[:])
            if B % 8 == 7:
                k = B // 8
                nc.sync.dma_start(
                    out=out_d[1024 * k: 1024 * (k + 1), :].rearrange(
                        "(b p) o -> p b o", p=128),
                    in_=obuf[:, 8 * k: 8 * k + 8, :],
                )

        # ---- interleaved schedule. C(B) production runs one row-chunk ahead
        # of the pair scatters so GPSIMD always has a 4-block basesT backlog;
        # emits lag pairs by one pair; D blocks are PE filler ----
        d_chunk(0, 8)
        conv1(0)
        conv1(1)
        conv2(0)
        for B in range(4):
            do_C(B)
        conv1(2)
        conv2(1)
        for B in range(4, 8):
            do_C(B)
        next_emit = [0]

        def emit_upto(maxB):
            while next_emit[0] <= maxB:
                emit_banded(next_emit[0])
                next_emit[0] += 1

        build_pair(1)   # pair 0: blocks 0,1
        build_pair(3)   # pair 1: blocks 2,3
        for r in range(2, 8):
            if r + 1 < 8:
                conv1(r + 1)
            conv2(r)
            for B in range(4 * r, 4 * r + 4):
                do_C(B)
            build_pair(4 * r - 3)   # pair 2r-2: blocks 4r-4, 4r-3
            emit_upto(4 * r - 5)
            build_pair(4 * r - 1)   # pair 2r-1: blocks 4r-2, 4r-1
            emit_upto(4 * r - 3)
            d_chunk(4 * r, 4 * r + 4)
        # tail: last two pairs + remaining emits
        build_pair(NBLK - 3)    # pair 14: blocks 28, 29
        emit_upto(27)
        build_pair(NBLK - 1)    # pair 15: blocks 30, 31
        emit_upto(NBLK - 1)

    nc.compile()
    return nc


def _get_program():
    if "nc" not in _cached:
        _cached["nc"] = _build_program()
    return _cached["nc"]


def _build_in_maps(inputs, prep=None):
    if prep is None:
        prep = _host_prep(inputs)
    x = np.asarray(inputs["x"], _f32)

    shared = {
        "w1t": np.ascontiguousarray(prep["w1T"].reshape(C, 9 * INTER)),
        "s1": prep["s1"], "t1": prep["t1"],
        "w2t": np.ascontiguousarray(prep["w2T"].reshape(INTER, 9 * BS)),
        "s2": prep["s2"], "t2": prep["t2"],
        "fbbd": prep["fbbd"],
        "coeft": np.ascontiguousarray(prep["coefT"].reshape(C, M * O)),
        "idx4": prep["idx4"],
        "ident": prep["ident"],
    }

    in_maps = []
    for n in range(N):
        m = dict(shared)
        m["xbf"] = np.ascontiguousarray(x[n].reshape(C, PIX).astype(_bf16))
        in_maps.append(m)
    return in_maps


def kernel(**inputs):
    from concourse.bass_utils import run_bass_kernel_spmd

    prep = _host_prep(inputs)
    in_maps = _build_in_maps(inputs, prep)

    nc = _get_program()
    res = run_bass_kernel_spmd(nc, in_maps, core_ids=list(range(N)))

    out = np.zeros((N, O, H, W), _f32)
    bias = prep["bias"]
    for n in range(N):
        outT = res.results[n]["out"]            # [4096, 128]
        out[n] = (outT.T + bias[:, None]).reshape(O, H, W)
    return out
